# revision 48
# baseline (speedup 1.0000x reference)
"""Trainium2 Bass kernel for nn_CrossAttention (B=8, Sq=Skv=2048, D=1024, C=768).

Strategy: data-parallel over batch — each of the 8 NeuronCores computes one
batch element's full cross-attention.

v2 design (vs v1 which staged K^T/Q^T via DRAM in f32r):
  * all matmul operands in bf16 — same PE throughput as f32r at free>=256
    (1 cycle/row) but half the DMA traffic and half the SBUF footprint.
  * K^T, V, Q^T and Wo live fully SBUF-resident; zero intermediate DRAM
    staging (v1 round-tripped 32 MB through HBM).
  * inputs stream in over TWO hardware DGE queues (sync + scalar) in
    compute-need order as a handful of large rearranged DMAs.
  * PE warmup matmuls at t=0 keep the tensor engine's p-state at full clock
    while the first operands land (idle PE restarts at 1.2 GHz for ~3 us).
  * softmax denominators accumulated across k-tiles on the DVE (16 tensor_adds)
    + one ones-matmul per q-block instead of 16 PE matmuls per q-block.

Per-core pipeline:
  phase 1a: K^T[d,k] = Wk @ ctx^T (+bk) -> SBUF bf16; V[k,d] = ctx @ Wv^T
            (+bv) -> SBUF bf16.  (per 512-wide k chunk)
  phase 1b: Q^T[d,q] = (Wq @ x^T + bq)/sqrt(D) -> SBUF bf16. (per 512 q chunk)
  phase 2 (per 512-wide q block):
      scores^T[k,q] = KT.T @ QT (accum over d)           -> PSUM
      expT = exp(scores^T) (ACT evac, no max: |s| small)  -> SBUF bf16
      partial[p,q] += expT[kt]  on DVE;  sums = ones.T @ partial (1 matmul)
      out^T[d,q] = V.T @ expT (accum over k)
      final[q,o] = outT.T @ WoT (accum over d); * 1/sums + bo; DMA out.

Softmax normalization commutes with the (linear) out-projection, so 1/sum is
applied on the final tiles where q sits on partitions.
"""

import numpy as np
import ml_dtypes

import concourse.bass as bass  # noqa: F401  (bass types used via bacc/tile)
import concourse.mybir as mybir
import concourse.tile as tile
from concourse import bacc
from concourse.bass_utils import run_bass_kernel_spmd

# ---- problem shapes (hardcoded) ----
B, SQ, SKV, D, C = 8, 2048, 2048, 1024, 768
P = 128
DT = D // P          # 8  d-tiles
CT = C // P          # 6  c-tiles
KT = SKV // P        # 16 k-tiles
QB = 512             # q block width
NQB = SQ // QB       # 4 q blocks
KC = 512             # k chunk width in phase 1a
NKC = SKV // KC      # 4
SCALE = 1.0 / np.sqrt(np.float32(D))
WARM_N = 8           # PE warmup matmuls covering the initial DMA window

F32 = mybir.dt.float32
F32R = mybir.dt.float32r
BF = mybir.dt.bfloat16
AF = mybir.ActivationFunctionType

_NC_CACHE = {}


def build():
    if "nc" in _NC_CACHE:
        return _NC_CACHE["nc"]
    nc = bacc.Bacc(trn_type="TRN2", num_swdge_queues=4)

    # ---- DRAM I/O (per-core slices; names = in_map keys) ----
    xT = nc.dram_tensor("xT", [D, SQ], BF, kind="ExternalInput")
    ctxT = nc.dram_tensor("ctxT", [C, SKV], BF, kind="ExternalInput")
    WqT = nc.dram_tensor("WqT", [D, D], BF, kind="ExternalInput")
    WkT = nc.dram_tensor("WkT", [C, D], BF, kind="ExternalInput")
    WvT = nc.dram_tensor("WvT", [C, D], BF, kind="ExternalInput")
    WoT = nc.dram_tensor("WoT", [D, D], BF, kind="ExternalInput")
    bqh = nc.dram_tensor("bqh", [P, DT], F32, kind="ExternalInput")  # bq*scale
    bkh = nc.dram_tensor("bkh", [P, DT], F32, kind="ExternalInput")
    bvb = nc.dram_tensor("bvb", [P, D], BF, kind="ExternalInput")    # bv bcast
    bob = nc.dram_tensor("bob", [P, D], F32, kind="ExternalInput")   # bo bcast
    onesmat = nc.dram_tensor("onesmat", [P, P], F32R, kind="ExternalInput")
    e0two = nc.dram_tensor("e0two", [P, 2], F32R, kind="ExternalInput")
    out = nc.dram_tensor("out", [SQ, D], F32, kind="ExternalOutput")

    with tile.TileContext(nc) as tc:
        with tc.tile_pool(name="persist", bufs=1) as persist:
            kt_sb = persist.tile([P, DT, SKV], BF, name="kt_sb")    # 32KB/p
            v_sb = persist.tile([P, KT, D], BF, name="v_sb")        # 32KB/p
            # Tile dependency tracking is per-TILE, so Q^T is one tile per
            # q-block: phase 2's first scores group then depends only on
            # its own block's evacs (one big tile made it wait for ALL of
            # phase 1b's last evac, a measured ~1.4us PE stall)
            qt_tiles = [persist.tile([P, DT, QB], BF, name=f"qt{qb}")
                        for qb in range(NQB)]                       # 32KB/p
            wo_sb = persist.tile([P, DT, D], BF, name="wo_sb")      # 16KB/p
            bq_sb = persist.tile([P, DT], F32, name="bq_sb")
            bk_sb = persist.tile([P, DT], F32, name="bk_sb")
            bv_sb = persist.tile([P, D], BF, name="bv_sb")
            bo_sb = persist.tile([P, D], F32, name="bo_sb")
            om_sb = persist.tile([P, P], F32R, name="om_sb")
            e0_sb = persist.tile([P, 2], F32R, name="e0_sb")
            warm_sb = persist.tile([P, 512], BF, name="warm_sb")
            sums_sb = persist.tile([P, QB], F32R, name="sums_sb")
            # consts go on the scalar HW queue, but AFTER ctx0 (emitted in
            # phase 1a below) — they're only needed at the first K evac

            # p1b pools opened early so x/wq prefetch DMAs can be issued
            # while phase 1a computes (SBUF: coexists with 1a pools)
            with tc.tile_pool(name="p1b_w", bufs=1) as p1b_w, \
                 tc.tile_pool(name="p1b_s", bufs=2) as p1b_s:
                wq_sb = p1b_w.tile([P, DT, D], BF, name="wq_sb")
                xt_tiles = [None] * NQB

                # ================= phase 1a: K^T + V resident =============
                # Section order K0 K1 V0 V1 K2 V2 K3 V3: the K sections only
                # need wk+ctx (the startup-gating loads), which gives the wv
                # halves ~20us to arrive behind them on the serial scalar
                # queue before the first V section needs them.
                with tc.tile_pool(name="p1a_w", bufs=1) as p1a_w, \
                     tc.tile_pool(name="p1a_s", bufs=2) as p1a_s, \
                     tc.tile_pool(name="ps_w", bufs=2, space="PSUM") as ps_w, \
                     tc.tile_pool(name="ps_k", bufs=4, space="PSUM") as ps_k, \
                     tc.tile_pool(name="ps_v", bufs=2, space="PSUM") as ps_v:
                    wk_sb = p1a_w.tile([P, CT, D], BF, name="wk_sb")
                    wv_sb = p1a_w.tile([P, CT, D], BF, name="wv_sb")

                    # PE warmup: keep the tensor engine busy (and its
                    # p-state at 2.4 GHz) while the first real operands
                    # stream in.
                    nc.gpsimd.memset(warm_sb[:], 0.0)
                    for _ in range(WARM_N):
                        pw = ps_w.tile([P, 512], F32, name="pw", tag="pw")
                        nc.tensor.matmul(pw, warm_sb[:, 0:P], warm_sb[:],
                                         start=True, stop=True)

                    # The DMA fabric is zero-sum (~350 GB/s shared, ~195
                    # GB/s per serial queue): every byte in flight early
                    # delays ctx0, the true gate.  So: exactly TWO streams,
                    # in strict need-order, sized so arrival tracks the
                    # PE's consumption.  wk goes in quarters so K group
                    # dt_i's weights land before the group runs.
                    #   sync:   wk_q0..q3, ctx1, ctx2, xt0, ctx3, xt1
                    #   scalar: ctx0, bkh, wv_h0, bvb, wv_h1, wq0, bqh,
                    #           wq1, om, e0, wo0, bob, wo1
                    # ctx0 split in two 256-wide HALF-TILES (separate tiles
                    # so per-tile dep tracking lets kc0's first K pass start
                    # after only ctx0a + wk_q0 = 0.75 MB in flight)
                    ctx0h = [p1a_w.tile([P, CT, 256], BF, name=f"ctx0{h}")
                             for h in range(2)]
                    ctx_tiles = [None] * NKC
                    for kc in range(1, 3):
                        ctx_tiles[kc] = p1a_s.tile([P, CT, KC], BF,
                                                   name="ctx_sb", tag="ctx")
                    nc.sync.dma_start(
                        wk_sb[:, :, 0:256],
                        WkT[:, 0:256].rearrange("(c p) d -> p c d", p=P))
                    for h in range(2):
                        nc.scalar.dma_start(
                            ctx0h[h][:],
                            ctxT[:, h * 256:(h + 1) * 256].rearrange(
                                "(c p) k -> p c k", p=P))
                    for quarter in range(1, 4):
                        nc.sync.dma_start(
                            wk_sb[:, :, quarter * 256:(quarter + 1) * 256],
                            WkT[:, quarter * 256:(quarter + 1) * 256]
                            .rearrange("(c p) d -> p c d", p=P))
                    nc.scalar.dma_start(bk_sb, bkh[:])
                    nc.scalar.dma_start(
                        wv_sb[:, :, 0:512],
                        WvT[:, 0:512].rearrange("(c p) d -> p c d", p=P))
                    nc.scalar.dma_start(bv_sb, bvb[:])
                    nc.scalar.dma_start(
                        wv_sb[:, :, 512:1024],
                        WvT[:, 512:1024].rearrange("(c p) d -> p c d", p=P))
                    nc.sync.dma_start(
                        ctx_tiles[1][:],
                        ctxT[:, KC:2 * KC].rearrange("(c p) k -> p c k", p=P))
                    nc.sync.dma_start(
                        ctx_tiles[2][:],
                        ctxT[:, 2 * KC:3 * KC].rearrange("(c p) k -> p c k",
                                                         p=P))

                    def emit_K(kc):
                        if kc == 0:
                            # two 256-wide passes (half-tile gating); psum
                            # groups use a 256 sub-range of the same [P,KC]
                            # pool tiles so the PSUM budget is unchanged
                            for h in range(2):
                                for dt_ in range(DT):
                                    pk = ps_k.tile([P, KC], F32, name="pk",
                                                   tag="pk")
                                    for ct_ in range(CT):
                                        nc.tensor.matmul(
                                            pk[:, 0:256],
                                            wk_sb[:, ct_,
                                                  dt_ * P:(dt_ + 1) * P],
                                            ctx0h[h][:, ct_, :],
                                            start=(ct_ == 0),
                                            stop=(ct_ == CT - 1))
                                    nc.scalar.activation(
                                        kt_sb[:, dt_,
                                              h * 256:(h + 1) * 256],
                                        pk[:, 0:256], AF.Identity,
                                        bias=bk_sb[:, dt_:dt_ + 1])
                            return
                        ctx_sb = ctx_tiles[kc]
                        for dt_ in range(DT):
                            pk = ps_k.tile([P, KC], F32, name="pk", tag="pk")
                            for ct_ in range(CT):
                                nc.tensor.matmul(
                                    pk, wk_sb[:, ct_, dt_ * P:(dt_ + 1) * P],
                                    ctx_sb[:, ct_, :],
                                    start=(ct_ == 0), stop=(ct_ == CT - 1))
                            nc.scalar.activation(
                                kt_sb[:, dt_, kc * KC:(kc + 1) * KC], pk,
                                AF.Identity, bias=bk_sb[:, dt_:dt_ + 1])

                    def emit_V(kc):
                        # dh-outer: the first 4 groups need only wv_h0,
                        # buying wv_h1 another ~5us of arrival slack
                        for dh in range(2):
                            for t in range(4):
                                kt_ = kc * 4 + t
                                if kc == 0:
                                    csrc = ctx0h[t // 2][:, :,
                                                         (t % 2) * P:
                                                         (t % 2 + 1) * P]
                                else:
                                    csrc = ctx_tiles[kc][:, :,
                                                         t * P:(t + 1) * P]
                                pv = ps_v.tile([P, 512], F32, name="pv",
                                               tag="pv")
                                for ct_ in range(CT):
                                    nc.tensor.matmul(
                                        pv, csrc[:, ct_],
                                        wv_sb[:, ct_, dh * 512:(dh + 1) * 512],
                                        start=(ct_ == 0), stop=(ct_ == CT - 1))
                                nc.vector.tensor_add(
                                    v_sb[:, kt_, dh * 512:(dh + 1) * 512],
                                    pv, bv_sb[:, dh * 512:(dh + 1) * 512])

                    emit_K(0)
                    nc.scalar.dma_start(
                        wq_sb[:, :, 0:512],
                        WqT[:, 0:512].rearrange("(i p) d -> p i d", p=P))
                    nc.scalar.dma_start(bq_sb, bqh[:])
                    emit_K(1)
                    nc.scalar.dma_start(
                        wq_sb[:, :, 512:1024],
                        WqT[:, 512:1024].rearrange("(i p) d -> p i d", p=P))
                    xt_tiles[0] = p1b_s.tile([P, DT, QB], BF,
                                             name="xt_sb", tag="xt")
                    nc.sync.dma_start(
                        xt_tiles[0][:],
                        xT[:, 0:QB].rearrange("(i p) q -> p i q", p=P))
                    emit_V(0)
                    nc.scalar.dma_start(om_sb, onesmat[:])
                    nc.scalar.dma_start(e0_sb, e0two[:])
                    nc.scalar.dma_start(
                        wo_sb[:, :, 0:512],
                        WoT[:, 0:512].rearrange("(i p) d -> p i d", p=P))
                    nc.scalar.dma_start(bo_sb, bob[:])
                    emit_V(1)
                    # ctx3 reuses ctx1's buffer (bufs=2); emitted only after
                    # V1 — ctx1's last reader — exists, so the rotation
                    # dependency covers it
                    ctx_tiles[3] = p1a_s.tile([P, CT, KC], BF,
                                              name="ctx_sb", tag="ctx")
                    nc.sync.dma_start(
                        ctx_tiles[3][:],
                        ctxT[:, 3 * KC:4 * KC].rearrange("(c p) k -> p c k",
                                                         p=P))
                    nc.scalar.dma_start(
                        wo_sb[:, :, 512:1024],
                        WoT[:, 512:1024].rearrange("(i p) d -> p i d", p=P))
                    xt_tiles[1] = p1b_s.tile([P, DT, QB], BF,
                                             name="xt_sb", tag="xt")
                    nc.sync.dma_start(
                        xt_tiles[1][:],
                        xT[:, QB:2 * QB].rearrange("(i p) q -> p i q", p=P))
                    emit_K(2)
                    emit_V(2)
                    emit_K(3)
                    emit_V(3)

                # ================= phase 1b: Q^T resident =================
                with tc.tile_pool(name="ps_q", bufs=2, space="PSUM") as ps_q:
                    for qb in range(NQB):
                        if xt_tiles[qb] is None:
                            xt_tiles[qb] = p1b_s.tile([P, DT, QB], BF,
                                                      name="xt_sb", tag="xt")
                            nc.sync.dma_start(
                                xt_tiles[qb][:],
                                xT[:, qb * QB:(qb + 1) * QB].rearrange(
                                    "(i p) q -> p i q", p=P))
                        xt_sb = xt_tiles[qb]
                        for dt_ in range(DT):
                            pq = ps_q.tile([P, QB], F32, name="pq", tag="pq")
                            for it in range(DT):
                                nc.tensor.matmul(
                                    pq, wq_sb[:, it, dt_ * P:(dt_ + 1) * P],
                                    xt_sb[:, it, :],
                                    start=(it == 0), stop=(it == DT - 1))
                            nc.scalar.activation(
                                qt_tiles[qb][:, dt_], pq,
                                AF.Identity, bias=bq_sb[:, dt_:dt_ + 1],
                                scale=float(SCALE))

            # ============== phase 2: attention + out proj =============
            with tc.tile_pool(name="p2_big", bufs=1) as p2_big, \
                 tc.tile_pool(name="p2_par", bufs=2) as p2_par, \
                 tc.tile_pool(name="p2_fin", bufs=4) as p2_fin, \
                 tc.tile_pool(name="p2_rcp", bufs=2) as p2_rcp, \
                 tc.tile_pool(name="ps_sc", bufs=2, space="PSUM") as ps_sc, \
                 tc.tile_pool(name="ps_sum", bufs=1, space="PSUM") as ps_sum, \
                 tc.tile_pool(name="ps_rt", bufs=1, space="PSUM") as ps_rt, \
                 tc.tile_pool(name="ps_out", bufs=2, space="PSUM") as ps_out, \
                 tc.tile_pool(name="ps_fin", bufs=2, space="PSUM") as ps_fin:
              for qb in range(NQB):
                # expt in two half-tiles: PV's first matmuls (kt 0-7) then
                # depend only on the first half's exps, which finished long
                # before, instead of on exp[15] (per-tile dep tracking)
                expt_ab = (p2_big.tile([P, KT // 2, QB], BF, name="expt_a",
                                       tag="expta"),
                           p2_big.tile([P, KT // 2, QB], BF, name="expt_b",
                                       tag="exptb"))
                def expt(kt):
                    return expt_ab[kt // 8][:, kt % 8]
                partial = p2_par.tile([P, QB], F32R, name="partial",
                                      tag="par")
                # ---- scores^T + exp + DVE partial-sum chain ----
                for kt_ in range(KT):
                    psc = ps_sc.tile([P, QB], F32, name="psc", tag="psc")
                    for dt_ in range(DT):
                        nc.tensor.matmul(
                            psc, kt_sb[:, dt_, kt_ * P:(kt_ + 1) * P],
                            qt_tiles[qb][:, dt_],
                            start=(dt_ == 0), stop=(dt_ == DT - 1))
                    nc.scalar.activation(expt(kt_), psc, AF.Exp)
                    if kt_ == 0:
                        nc.vector.tensor_copy(partial, expt(0))
                    else:
                        nc.vector.tensor_add(partial, partial, expt(kt_))
                # ---- out^T = V.T @ expT (d-quarter passes) ----
                # outt split in halves for the same per-tile-dep reason:
                # the final projection's first matmuls (dt 0-3) then don't
                # wait for the dp3 evacuation
                outt_ab = (p2_big.tile([P, DT // 2, QB], BF, name="outt_a",
                                       tag="outta"),
                           p2_big.tile([P, DT // 2, QB], BF, name="outt_b",
                                       tag="outtb"))
                def outt(dt):
                    return outt_ab[dt // 4][:, dt % 4]
                for dp in range(4):
                    po0 = ps_out.tile([P, QB], F32, name="po0", tag="po")
                    po1 = ps_out.tile([P, QB], F32, name="po1", tag="po")
                    po = (po0, po1)
                    for kt_ in range(KT):
                        for dc in range(2):
                            d0 = dp * 256 + dc * P
                            nc.tensor.matmul(
                                po[dc], v_sb[:, kt_, d0:d0 + P],
                                expt(kt_),
                                start=(kt_ == 0), stop=(kt_ == KT - 1))
                    for dc in range(2):
                        nc.scalar.copy(outt(dp * 2 + dc), po[dc])
                # ---- sums over partitions (1 matmul) + 1/sums on q ----
                # (emitted after PV so the exp[15] -> DVE-chain latency
                # hides under the PV matmuls instead of stalling the PE)
                psums = ps_sum.tile([P, QB], F32, name="psums", tag="psums")
                nc.tensor.matmul(psums, om_sb, partial, start=True, stop=True)
                nc.scalar.copy(sums_sb, psums)
                prt = ps_rt.tile([P, 8], F32, name="prt", tag="prt")
                for qs in range(4):
                    nc.tensor.matmul(
                        prt[:, 2 * qs:2 * qs + 2],
                        sums_sb[:, qs * P:(qs + 1) * P], e0_sb,
                        start=True, stop=True)
                recip = p2_rcp.tile([P, 8], F32, name="recip", tag="recip")
                nc.vector.reciprocal(recip, prt)
                # ---- final = out^T.T @ WoT, * 1/sums + bo ----
                for qs in range(4):
                    for oc in range(2):
                        pf = ps_fin.tile([P, 512], F32, name="pf", tag="pf")
                        for dt_ in range(DT):
                            nc.tensor.matmul(
                                pf, outt(dt_)[:, qs * P:(qs + 1) * P],
                                wo_sb[:, dt_, oc * 512:(oc + 1) * 512],
                                start=(dt_ == 0), stop=(dt_ == DT - 1))
                        fin = p2_fin.tile([P, 512], F32, name="fin",
                                          tag="fin")
                        # outputs on the sync HW queue (idle in phase 2;
                        # the gpsimd SW queue made the final write the
                        # kernel's tail).  The very last chunk is evacuated
                        # in 256-wide halves so its trailing
                        # mul+add+DMA chain after the final matmul is half
                        # as long.
                        last = (qb == NQB - 1 and qs == 3 and oc == 1)
                        for h0, hw in ([(0, 256), (256, 256)] if last
                                       else [(0, 512)]):
                            nc.scalar.mul(fin[:, h0:h0 + hw],
                                          pf[:, h0:h0 + hw],
                                          recip[:, 2 * qs:2 * qs + 1])
                            nc.vector.tensor_add(
                                fin[:, h0:h0 + hw], fin[:, h0:h0 + hw],
                                bo_sb[:, oc * 512 + h0:oc * 512 + h0 + hw])
                            nc.sync.dma_start(
                                out[qb * QB + qs * P: qb * QB + (qs + 1) * P,
                                    oc * 512 + h0:oc * 512 + h0 + hw],
                                fin[:, h0:h0 + hw])
    nc.finalize()
    _NC_CACHE["nc"] = nc
    return nc


def _host_prep(x, context, Wq, bq, Wk, bk, Wv, bv, Wo, bo):
    """Build the 8 per-core input maps (host-side layout prep)."""
    bf = ml_dtypes.bfloat16
    x = np.asarray(x, dtype=np.float32)
    context = np.asarray(context, dtype=np.float32)
    WqT = np.ascontiguousarray(np.asarray(Wq, np.float32).T).astype(bf)
    WkT = np.ascontiguousarray(np.asarray(Wk, np.float32).T).astype(bf)
    WvT = np.ascontiguousarray(np.asarray(Wv, np.float32).T).astype(bf)
    WoT = np.ascontiguousarray(np.asarray(Wo, np.float32).T).astype(bf)
    scale = np.float32(1.0 / np.sqrt(np.float32(D)))
    bqh = np.ascontiguousarray(
        (np.asarray(bq, np.float32) * scale).reshape(DT, P).T)  # [p, dt]
    bkh = np.ascontiguousarray(np.asarray(bk, np.float32).reshape(DT, P).T)
    bvb = np.ascontiguousarray(
        np.broadcast_to(np.asarray(bv, np.float32)[None, :],
                        (P, D))).astype(bf)
    bob = np.ascontiguousarray(
        np.broadcast_to(np.asarray(bo, np.float32)[None, :], (P, D)))
    onesmat = np.ones((P, P), np.float32)
    e0two = np.zeros((P, 2), np.float32)
    e0two[0, :] = 1.0
    shared = dict(WqT=WqT, WkT=WkT, WvT=WvT, WoT=WoT, bqh=bqh, bkh=bkh,
                  bvb=bvb, bob=bob, onesmat=onesmat, e0two=e0two)
    in_maps = []
    for b in range(B):
        m = dict(shared)
        m["xT"] = np.ascontiguousarray(x[b].T).astype(bf)        # [D, SQ]
        m["ctxT"] = np.ascontiguousarray(context[b].T).astype(bf)  # [C, SKV]
        in_maps.append(m)
    return in_maps


def kernel(**inputs) -> np.ndarray:
    nc = build()
    in_maps = _host_prep(**inputs)
    res = run_bass_kernel_spmd(nc, in_maps, core_ids=list(range(B)))
    return np.stack([res.results[b]["out"] for b in range(B)], axis=0)
